# revision 16
# baseline (speedup 1.0000x reference)
"""Cross-stitch unit kernel for Trainium2 (8 NeuronCores, data-parallel).

Computes, per channel c:
  out_a[n,c,h,w] = w[c,0,0]*x_a[n,c,h,w] + w[c,0,1]*x_b[n,c,h,w]
  out_b[n,c,h,w] = w[c,1,0]*x_a[n,c,h,w] + w[c,1,1]*x_b[n,c,h,w]

Sharding: batch dim (N=32) split 4-per-core across 8 cores; weights
replicated. The kernel is DMA-bound (SBUF-fabric ceiling ~435 GB/s/core)
so all device I/O is fp16 (inputs quantized on host, rel err ~2^-11,
far inside the 2e-2 gate); traffic is 33.55 MB/core -> ~79 us roofline.

The whole 2x2 per-channel mix runs on the OTHERWISE-IDLE tensor engine
as a block-diagonal matmul (DVE scalar_tensor_tensor is 1x in 16-bit,
and ScalarE has no 16-bit fast mode, so elementwise formulations leave
one of those engines as the co-bottleneck):

  partitions of a tile = (stream i, 64 channels); lhsT = W_g [128,128]
  fp16 with W_g[i*64+c, o*64+c] = w[g*64+c, o, i] (block-diag, built on
  host); one matmul contracts over partitions: out[(o,c), f] =
  sum_i w[c,o,i] x[(i,c), f] -- all 4 muls + 2 adds, f32 PSUM accum
  (better precision: one fp16 rounding at the PSUM->SBUF cast).

Everything runs at 0.5 MiB sub-iteration granularity v (32 subs; tile
t = v//2 is a [128, 4096] block, g-major so the stationary W_g is
stable across 8 consecutive subs). PSUM is double-buffered: sub v uses
banks (v%2)*4..+4 (a matmul can target at most ONE bank: FD>512 fails
walrus codegen). PE(v) waits for the casts of v-2, not v-1 -- a full
sub-iteration of slack absorbs semaphore latencies. The tiny weights
DMAs go FIRST on the SP ring: on the ACT ring they queue behind ~4 MiB
of input-load packets and stall the first matmul by ~14 us (observed).

Per sub-iteration (DMA budget 2.46 us):
  SP  : weights x4 up front; then one 0.5 MiB load  -> s_w / s_load
  PE  : 4 matmuls of [128,512] (~0.4 us each incl per-matmul LDWEIGHTS;
        ldw-opt is disabled in this walrus)          -> s_mm  (+1)
  DVE : cast PSUM[h, 0:1024] -> SBUF fp16 (1.2 us)   -> s_cpa (+1)
  ACT : cast PSUM[h, 1024:2048] (1.0 us) + store DMA -> s_cpb / s_store
The PSUM column split is bank-aligned: a mid-bank engine read of PSUM
hangs the device (observed with a 1280/768 split).
All engines <=80% of the DMA budget -> cleanly DMA-bound.

Raw Bass (no Tile): the installed walrus codegen accepts at most ONE
sync-wait per instruction; every cross-engine dependency below is a
single standalone wait_ge. NOTE the engines run in relaxed ordering
mode: same-engine program order does NOT order a compute instruction's
data effects before a later DMA trigger, so the store waits on s_cpb
(ACT's own cast) explicitly -- without it the store DMA reads o_sb
mid-cast (observed on HW).
"""

import numpy as np

import concourse.bass as bass
import concourse.mybir as mybir
from concourse.bass_utils import run_bass_kernel_spmd

N, C, H, W = 32, 256, 64, 64
N_CORES = 8
N_LOC = N // N_CORES          # 4 images per core
F = H * W                     # 4096 elements per (n, c) row
P = 128                       # SBUF partitions
G = C // 64                   # 4 channel groups of 64 (x 2 streams = 128)
N_TILES = N_LOC * G           # 16 layout tiles per core, T = g*N_LOC + n
N_SUB = 2 * N_TILES           # 32 sub-iterations (0.5 MiB each)
B = 8                         # SBUF slot buffering (tiles): x+o = 128 KB/partition
CHUNK = 512                   # matmul free size = one PSUM bank (f32)
HALF = F // 2                 # 2048 cols per sub-iteration
SPLIT = 1024                  # DVE casts [0:SPLIT), ACT casts [SPLIT:HALF)
                              # (PSUM-bank-aligned: mid-bank reads hang)

FP16 = np.float16

_nc_cache = {}


def _build():
    if "nc" in _nc_cache:
        return _nc_cache["nc"]

    nc = bass.Bass()
    dt = mybir.dt.float16
    f32 = mybir.dt.float32
    x_d = nc.declare_dram_parameter("x", [N_TILES, P, F], dt, isOutput=False)
    # Host-built block-diagonal mix matrices, one per 64-channel group.
    wts = nc.declare_dram_parameter("weights", [G, P, P], dt, isOutput=False)
    out_d = nc.declare_dram_parameter("out", [N_TILES, P, F], dt, isOutput=True)

    with (
        nc.sbuf_tensor([P, B, F], dt) as x_sb,
        nc.sbuf_tensor([P, B, F], dt) as o_sb,
        nc.sbuf_tensor([P, G, P], dt) as w_sb,
        nc.psum_tensor("ps", [P, 2, HALF], f32) as ps,
        nc.semaphore("s_load") as s_load,
        nc.semaphore("s_w") as s_w,
        nc.semaphore("s_mm") as s_mm,
        nc.semaphore("s_cpa") as s_cpa,
        nc.semaphore("s_cpb") as s_cpb,
        nc.semaphore("s_store") as s_store,
        nc.Block() as block,
    ):

        @block.sync
        def _(sync):
            for g in range(G):
                sync.dma_start(out=w_sb[:, g], in_=wts[g]).then_inc(s_w, 16)
            for t in range(N_TILES):
                s = t % B
                if t >= B:
                    # WAR: PE finished reading x slot t-B (both subs).
                    sync.wait_ge(s_mm, 2 * (t - B) + 2)
                sync.dma_start(
                    out=x_sb[:, s], in_=x_d[t]
                ).then_inc(s_load, 16)

        @block.tensor
        def _(tensor):
            tensor.wait_ge(s_w, 16 * G)
            for v in range(N_SUB):
                t, h = v // 2, v % 2
                g = t // N_LOC
                s = t % B
                if h == 0:
                    # RAW: load(t) landed.
                    tensor.wait_ge(s_load, 16 * (t + 1))
                if v >= 2:
                    # WAR: PSUM half h of sub v-2 fully cast out.
                    tensor.wait_ge(s_cpa, v - 1)
                    tensor.wait_ge(s_cpb, v - 1)
                for k in range(HALF // CHUNK):
                    mm = tensor.matmul(
                        ps[:, h, k * CHUNK:(k + 1) * CHUNK],
                        w_sb[:, g],
                        x_sb[:, s, h * HALF + k * CHUNK:
                                   h * HALF + (k + 1) * CHUNK],
                        start=True,
                        stop=True,
                    )
                    if k == HALF // CHUNK - 1:
                        mm.then_inc(s_mm, 1)

        @block.vector
        def _(vector):
            for v in range(N_SUB):
                t, h = v // 2, v % 2
                s = t % B
                # RAW: PE sub v done.
                vector.wait_ge(s_mm, v + 1)
                if h == 0 and t >= B:
                    # WAR: store(t-B) done reading this o slot.
                    vector.wait_ge(s_store, 16 * (t - B + 1))
                vector.tensor_copy(
                    out=o_sb[:, s, h * HALF:h * HALF + SPLIT],
                    in_=ps[:, h, 0:SPLIT],
                ).then_inc(s_cpa, 1)

        @block.scalar
        def _(scalar):
            for v in range(N_SUB):
                t, h = v // 2, v % 2
                s = t % B
                # RAW: PE sub v done.
                scalar.wait_ge(s_mm, v + 1)
                if h == 0 and t >= B:
                    # WAR: store(t-B) done reading this o slot.
                    scalar.wait_ge(s_store, 16 * (t - B + 1))
                scalar.copy(
                    o_sb[:, s, h * HALF + SPLIT:(h + 1) * HALF],
                    ps[:, h, SPLIT:HALF],
                ).then_inc(s_cpb, 1)
                if t == N_TILES - 1:
                    # Last tile: store each half as soon as its casts land
                    # (trims the serial mm->cast->store drain tail ~1.2us).
                    scalar.wait_ge(s_cpa, v + 1)
                    scalar.wait_ge(s_cpb, v + 1)
                    scalar.dma_start(
                        out=out_d[t, :, h * HALF:(h + 1) * HALF],
                        in_=o_sb[:, s, h * HALF:(h + 1) * HALF],
                    ).then_inc(s_store, 16)
                elif h == 1:
                    # RAW: all four casts of tile t done. Relaxed ordering:
                    # ACT's own casts above need the s_cpb wait too.
                    scalar.wait_ge(s_cpa, v + 1)
                    scalar.wait_ge(s_cpb, v + 1)
                    scalar.dma_start(
                        out=out_d[t], in_=o_sb[:, s]
                    ).then_inc(s_store, 16)

    _nc_cache["nc"] = nc
    return nc


def run_sharded(x_a, x_b, weights, **spmd_kwargs):
    """Shard, run on 8 cores, gather. Returns ((out_a, out_b), BassKernelResults)."""
    nc = _build()
    # Cast to fp16 FIRST (halves the bytes the interleave/transpose moves).
    # Tile t = g*N_LOC + n holds [(stream i, 64 channels), F] for channel
    # group g: partition p = i*64 + c_lo.
    xa = np.asarray(x_a).astype(FP16).reshape(N_CORES, N_LOC, G, 64, F)
    xb = np.asarray(x_b).astype(FP16).reshape(N_CORES, N_LOC, G, 64, F)
    x = np.stack([xa, xb], axis=3)              # [cores, n, g, i, 64, F]
    x = np.ascontiguousarray(
        x.transpose(0, 2, 1, 3, 4, 5)           # [cores, g, n, i, 64, F]
        .reshape(N_CORES, N_TILES, P, F)
    )
    # Block-diagonal mix matrices: W[g, i*64+c, o*64+c] = w[g*64+c, o, i].
    w = np.asarray(weights, dtype=np.float32).reshape(G, 64, 2, 2)
    wmat = np.zeros((G, P, P), dtype=np.float32)
    idx = np.arange(64)
    for i in range(2):
        for o in range(2):
            wmat[:, i * 64 + idx, o * 64 + idx] = w[:, idx, o, i]
    wmat = wmat.astype(FP16)
    in_maps = [{"x": x[i], "weights": wmat} for i in range(N_CORES)]
    res = run_bass_kernel_spmd(nc, in_maps, list(range(N_CORES)), **spmd_kwargs)
    out = np.stack([res.results[i]["out"] for i in range(N_CORES)])
    # out[core, t=(g,n), p=(o, c_lo), F] -> [2, N, C, H, W], upcast f32.
    out = out.reshape(N_CORES, G, N_LOC, 2, 64, F).astype(np.float32)
    out = out.transpose(3, 0, 2, 1, 4, 5).reshape(2, N, C, H, W)
    return (out[0], out[1]), res


def kernel(x_a, x_b, weights):
    (out_a, out_b), _ = run_sharded(x_a, x_b, weights)
    return out_a, out_b


# revision 17
# speedup vs baseline: 1.0030x; 1.0030x over previous
"""Cross-stitch unit kernel for Trainium2 (8 NeuronCores, data-parallel).

Computes, per channel c:
  out_a[n,c,h,w] = w[c,0,0]*x_a[n,c,h,w] + w[c,0,1]*x_b[n,c,h,w]
  out_b[n,c,h,w] = w[c,1,0]*x_a[n,c,h,w] + w[c,1,1]*x_b[n,c,h,w]

Sharding: batch dim (N=32) split 4-per-core across 8 cores; weights
replicated. The kernel is DMA-bound (SBUF-fabric ceiling ~435 GB/s/core)
so all device I/O is fp16 (inputs quantized on host, rel err ~2^-11,
far inside the 2e-2 gate); traffic is 33.55 MB/core -> ~79 us roofline.

The whole 2x2 per-channel mix runs on the OTHERWISE-IDLE tensor engine
as a block-diagonal matmul (DVE scalar_tensor_tensor is 1x in 16-bit,
and ScalarE has no 16-bit fast mode, so elementwise formulations leave
one of those engines as the co-bottleneck):

  partitions of a tile = (stream i, 64 channels); lhsT = W_g [128,128]
  fp16 with W_g[i*64+c, o*64+c] = w[g*64+c, o, i] (block-diag, built on
  host); one matmul contracts over partitions: out[(o,c), f] =
  sum_i w[c,o,i] x[(i,c), f] -- all 4 muls + 2 adds, f32 PSUM accum
  (better precision: one fp16 rounding at the PSUM->SBUF cast).

Everything runs at 0.5 MiB sub-iteration granularity v (32 subs; tile
t = v//2 is a [128, 4096] block, g-major so the stationary W_g is
stable across 8 consecutive subs). PSUM is double-buffered: sub v uses
banks (v%2)*4..+4 (a matmul can target at most ONE bank: FD>512 fails
walrus codegen). PE(v) waits for the casts of v-2, not v-1 -- a full
sub-iteration of slack absorbs semaphore latencies. The tiny weights
DMAs go FIRST on the SP ring: on the ACT ring they queue behind ~4 MiB
of input-load packets and stall the first matmul by ~14 us (observed).

Per sub-iteration (DMA budget 2.46 us):
  SP  : weights x4 up front; then one 0.5 MiB load  -> s_w / s_load
  PE  : 4 matmuls of [128,512] (~0.4 us each incl per-matmul LDWEIGHTS;
        ldw-opt is disabled in this walrus)          -> s_mm  (+1)
  DVE : cast PSUM[h, 0:1024] -> SBUF fp16 (1.2 us)   -> s_cpa (+1)
  ACT : cast PSUM[h, 1024:2048] (1.0 us) + store DMA -> s_cpb / s_store
The PSUM column split is bank-aligned: a mid-bank engine read of PSUM
hangs the device (observed with a 1280/768 split).
All engines <=80% of the DMA budget -> cleanly DMA-bound.

Raw Bass (no Tile): the installed walrus codegen accepts at most ONE
sync-wait per instruction; every cross-engine dependency below is a
single standalone wait_ge. NOTE the engines run in relaxed ordering
mode: same-engine program order does NOT order a compute instruction's
data effects before a later DMA trigger, so the store waits on s_cpb
(ACT's own cast) explicitly -- without it the store DMA reads o_sb
mid-cast (observed on HW).
"""

import numpy as np

import concourse.bass as bass
import concourse.mybir as mybir
from concourse.bass_utils import run_bass_kernel_spmd

N, C, H, W = 32, 256, 64, 64
N_CORES = 8
N_LOC = N // N_CORES          # 4 images per core
F = H * W                     # 4096 elements per (n, c) row
P = 128                       # SBUF partitions
G = C // 64                   # 4 channel groups of 64 (x 2 streams = 128)
N_TILES = N_LOC * G           # 16 layout tiles per core, T = g*N_LOC + n
N_SUB = 2 * N_TILES           # 32 sub-iterations (0.5 MiB each)
B = 5                         # SBUF slot buffering (tiles): x+o = 80 KB/partition
CHUNK = 512                   # matmul free size = one PSUM bank (f32)
HALF = F // 2                 # 2048 cols per sub-iteration
SPLIT = 1024                  # DVE casts [0:SPLIT), ACT casts [SPLIT:HALF)
                              # (PSUM-bank-aligned: mid-bank reads hang)

FP16 = np.float16

_nc_cache = {}


def _build():
    if "nc" in _nc_cache:
        return _nc_cache["nc"]

    nc = bass.Bass()
    dt = mybir.dt.float16
    f32 = mybir.dt.float32
    x_d = nc.declare_dram_parameter("x", [N_TILES, P, F], dt, isOutput=False)
    # Host-built block-diagonal mix matrices, one per 64-channel group.
    wts = nc.declare_dram_parameter("weights", [G, P, P], dt, isOutput=False)
    out_d = nc.declare_dram_parameter("out", [N_TILES, P, F], dt, isOutput=True)

    with (
        nc.sbuf_tensor([P, B, F], dt) as x_sb,
        nc.sbuf_tensor([P, B, F], dt) as o_sb,
        nc.sbuf_tensor([P, G, P], dt) as w_sb,
        nc.psum_tensor("ps", [P, 2, HALF], f32) as ps,
        nc.semaphore("s_load") as s_load,
        nc.semaphore("s_w") as s_w,
        nc.semaphore("s_mm") as s_mm,
        nc.semaphore("s_cpa") as s_cpa,
        nc.semaphore("s_cpb") as s_cpb,
        nc.semaphore("s_store") as s_store,
        nc.Block() as block,
    ):

        @block.sync
        def _(sync):
            for g in range(G):
                sync.dma_start(out=w_sb[:, g], in_=wts[g]).then_inc(s_w, 16)
            for t in range(N_TILES):
                s = t % B
                if t >= B:
                    # WAR: PE finished reading x slot t-B (both subs).
                    sync.wait_ge(s_mm, 2 * (t - B) + 2)
                sync.dma_start(
                    out=x_sb[:, s], in_=x_d[t]
                ).then_inc(s_load, 16)

        @block.tensor
        def _(tensor):
            tensor.wait_ge(s_w, 16 * G)
            for v in range(N_SUB):
                t, h = v // 2, v % 2
                g = t // N_LOC
                s = t % B
                if h == 0:
                    # RAW: load(t) landed.
                    tensor.wait_ge(s_load, 16 * (t + 1))
                if v >= 2:
                    # WAR: PSUM half h of sub v-2 fully cast out.
                    tensor.wait_ge(s_cpa, v - 1)
                    tensor.wait_ge(s_cpb, v - 1)
                for k in range(HALF // CHUNK):
                    mm = tensor.matmul(
                        ps[:, h, k * CHUNK:(k + 1) * CHUNK],
                        w_sb[:, g],
                        x_sb[:, s, h * HALF + k * CHUNK:
                                   h * HALF + (k + 1) * CHUNK],
                        start=True,
                        stop=True,
                    )
                    if k == HALF // CHUNK - 1:
                        mm.then_inc(s_mm, 1)

        @block.vector
        def _(vector):
            for v in range(N_SUB):
                t, h = v // 2, v % 2
                s = t % B
                # RAW: PE sub v done.
                vector.wait_ge(s_mm, v + 1)
                if h == 0 and t >= B:
                    # WAR: store(t-B) done reading this o slot.
                    vector.wait_ge(s_store, 16 * (t - B + 1))
                vector.tensor_copy(
                    out=o_sb[:, s, h * HALF:h * HALF + SPLIT],
                    in_=ps[:, h, 0:SPLIT],
                ).then_inc(s_cpa, 1)

        @block.scalar
        def _(scalar):
            for v in range(N_SUB):
                t, h = v // 2, v % 2
                s = t % B
                # RAW: PE sub v done.
                scalar.wait_ge(s_mm, v + 1)
                if h == 0 and t >= B:
                    # WAR: store(t-B) done reading this o slot.
                    scalar.wait_ge(s_store, 16 * (t - B + 1))
                scalar.copy(
                    o_sb[:, s, h * HALF + SPLIT:(h + 1) * HALF],
                    ps[:, h, SPLIT:HALF],
                ).then_inc(s_cpb, 1)
                if t == N_TILES - 1:
                    # Last tile: store each half as soon as its casts land
                    # (trims the serial mm->cast->store drain tail ~1.2us).
                    scalar.wait_ge(s_cpa, v + 1)
                    scalar.wait_ge(s_cpb, v + 1)
                    scalar.dma_start(
                        out=out_d[t, :, h * HALF:(h + 1) * HALF],
                        in_=o_sb[:, s, h * HALF:(h + 1) * HALF],
                    ).then_inc(s_store, 16)
                elif h == 1:
                    # RAW: all four casts of tile t done. Relaxed ordering:
                    # ACT's own casts above need the s_cpb wait too.
                    scalar.wait_ge(s_cpa, v + 1)
                    scalar.wait_ge(s_cpb, v + 1)
                    scalar.dma_start(
                        out=out_d[t], in_=o_sb[:, s]
                    ).then_inc(s_store, 16)

    _nc_cache["nc"] = nc
    return nc


def run_sharded(x_a, x_b, weights, **spmd_kwargs):
    """Shard, run on 8 cores, gather. Returns ((out_a, out_b), BassKernelResults)."""
    nc = _build()
    # Cast to fp16 FIRST (halves the bytes the interleave/transpose moves).
    # Tile t = g*N_LOC + n holds [(stream i, 64 channels), F] for channel
    # group g: partition p = i*64 + c_lo.
    xa = np.asarray(x_a).astype(FP16).reshape(N_CORES, N_LOC, G, 64, F)
    xb = np.asarray(x_b).astype(FP16).reshape(N_CORES, N_LOC, G, 64, F)
    x = np.stack([xa, xb], axis=3)              # [cores, n, g, i, 64, F]
    x = np.ascontiguousarray(
        x.transpose(0, 2, 1, 3, 4, 5)           # [cores, g, n, i, 64, F]
        .reshape(N_CORES, N_TILES, P, F)
    )
    # Block-diagonal mix matrices: W[g, i*64+c, o*64+c] = w[g*64+c, o, i].
    w = np.asarray(weights, dtype=np.float32).reshape(G, 64, 2, 2)
    wmat = np.zeros((G, P, P), dtype=np.float32)
    idx = np.arange(64)
    for i in range(2):
        for o in range(2):
            wmat[:, i * 64 + idx, o * 64 + idx] = w[:, idx, o, i]
    wmat = wmat.astype(FP16)
    in_maps = [{"x": x[i], "weights": wmat} for i in range(N_CORES)]
    res = run_bass_kernel_spmd(nc, in_maps, list(range(N_CORES)), **spmd_kwargs)
    out = np.stack([res.results[i]["out"] for i in range(N_CORES)])
    # out[core, t=(g,n), p=(o, c_lo), F] -> [2, N, C, H, W], upcast f32.
    out = out.reshape(N_CORES, G, N_LOC, 2, 64, F).astype(np.float32)
    out = out.transpose(3, 0, 2, 1, 4, 5).reshape(2, N, C, H, W)
    return (out[0], out[1]), res


def kernel(x_a, x_b, weights):
    (out_a, out_b), _ = run_sharded(x_a, x_b, weights)
    return out_a, out_b


# revision 18
# speedup vs baseline: 1.0469x; 1.0438x over previous
"""Cross-stitch unit kernel for Trainium2 (8 NeuronCores, data-parallel).

Computes, per channel c:
  out_a[n,c,h,w] = w[c,0,0]*x_a[n,c,h,w] + w[c,0,1]*x_b[n,c,h,w]
  out_b[n,c,h,w] = w[c,1,0]*x_a[n,c,h,w] + w[c,1,1]*x_b[n,c,h,w]

Sharding: batch dim (N=32) split 4-per-core across 8 cores; weights
replicated. The kernel is DMA-bound (SBUF-fabric ceiling ~435 GB/s/core)
so all device I/O is fp16 (inputs quantized on host, rel err ~2^-11,
far inside the 2e-2 gate); traffic is 33.55 MB/core -> ~79 us roofline.

The whole 2x2 per-channel mix runs on the OTHERWISE-IDLE tensor engine
as a block-diagonal matmul (DVE scalar_tensor_tensor is 1x in 16-bit,
and ScalarE has no 16-bit fast mode, so elementwise formulations leave
one of those engines as the co-bottleneck):

  partitions of a tile = (stream i, 64 channels); lhsT = W_g [128,128]
  fp16 with W_g[i*64+c, o*64+c] = w[g*64+c, o, i] (block-diag, built on
  host); one matmul contracts over partitions: out[(o,c), f] =
  sum_i w[c,o,i] x[(i,c), f] -- all 4 muls + 2 adds, f32 PSUM accum
  (better precision: one fp16 rounding at the PSUM->SBUF cast).

Everything runs at 0.5 MiB sub-iteration granularity v (32 subs; tile
t = v//2 is a [128, 4096] block, g-major so the stationary W_g is
stable across 8 consecutive subs). PSUM is double-buffered: sub v uses
banks (v%2)*4..+4 (a matmul can target at most ONE bank: FD>512 fails
walrus codegen). PE(v) waits for the casts of v-2, not v-1 -- a full
sub-iteration of slack absorbs semaphore latencies. The tiny weights
DMAs go FIRST on the SP ring: on the ACT ring they queue behind ~4 MiB
of input-load packets and stall the first matmul by ~14 us (observed).

Per sub-iteration (DMA budget 2.46 us):
  SP  : weights x4 up front; then one 0.5 MiB load  -> s_w / s_load
  PE  : 4 matmuls of [128,512] (~0.4 us each incl per-matmul LDWEIGHTS;
        ldw-opt is disabled in this walrus)          -> s_mm  (+1)
  DVE : cast PSUM[h, 0:1024] -> SBUF fp16 (1.2 us)   -> s_cpa (+1)
  ACT : cast PSUM[h, 1024:2048] (1.0 us) + store DMA -> s_cpb / s_store
The PSUM column split is bank-aligned: a mid-bank engine read of PSUM
hangs the device (observed with a 1280/768 split).
All engines <=80% of the DMA budget -> cleanly DMA-bound.

Raw Bass (no Tile): the installed walrus codegen accepts at most ONE
sync-wait per instruction; every cross-engine dependency below is a
single standalone wait_ge. NOTE the engines run in relaxed ordering
mode: same-engine program order does NOT order a compute instruction's
data effects before a later DMA trigger, so the store waits on s_cpb
(ACT's own cast) explicitly -- without it the store DMA reads o_sb
mid-cast (observed on HW).
"""

import numpy as np

import concourse.bass as bass
import concourse.mybir as mybir
from concourse.bass_utils import run_bass_kernel_spmd

N, C, H, W = 32, 256, 64, 64
N_CORES = 8
N_LOC = N // N_CORES          # 4 images per core
F = H * W                     # 4096 elements per (n, c) row
P = 128                       # SBUF partitions
G = C // 64                   # 4 channel groups of 64 (x 2 streams = 128)
N_TILES = N_LOC * G           # 16 layout tiles per core, T = g*N_LOC + n
N_SUB = 2 * N_TILES           # 32 sub-iterations (0.5 MiB each)
B = 5                         # SBUF slot buffering (tiles): x+o = 80 KB/partition
CHUNK = 512                   # matmul free size = one PSUM bank (f32)
HALF = F // 2                 # 2048 cols per sub-iteration
SPLIT = 1024                  # DVE casts [0:SPLIT), ACT casts [SPLIT:HALF)
                              # (PSUM-bank-aligned: mid-bank reads hang)

FP16 = np.float16

_nc_cache = {}


def _build():
    if "nc" in _nc_cache:
        return _nc_cache["nc"]

    nc = bass.Bass()
    dt = mybir.dt.float16
    f32 = mybir.dt.float32
    x_d = nc.declare_dram_parameter("x", [N_TILES, P, F], dt, isOutput=False)
    # Host-built block-diagonal mix matrices, one per 64-channel group.
    wts = nc.declare_dram_parameter("weights", [G, P, P], dt, isOutput=False)
    out_d = nc.declare_dram_parameter("out", [N_TILES, P, F], dt, isOutput=True)

    with (
        nc.sbuf_tensor([P, B, F], dt) as x_sb,
        nc.sbuf_tensor([P, B, F], dt) as o_sb,
        nc.sbuf_tensor([P, G, P], dt) as w_sb,
        nc.psum_tensor("ps", [P, 2, HALF], f32) as ps,
        nc.semaphore("s_load") as s_load,
        nc.semaphore("s_w") as s_w,
        nc.semaphore("s_mm") as s_mm,
        nc.semaphore("s_cpa") as s_cpa,
        nc.semaphore("s_cpb") as s_cpb,
        nc.semaphore("s_store") as s_store,
        nc.Block() as block,
    ):

        @block.sync
        def _(sync):
            for g in range(G):
                sync.dma_start(out=w_sb[:, g], in_=wts[g]).then_inc(s_w, 16)
            for t in range(N_TILES):
                s = t % B
                if t >= B:
                    # WAR: PE finished reading x slot t-B (both subs).
                    sync.wait_ge(s_mm, 2 * (t - B) + 2)
                sync.dma_start(
                    out=x_sb[:, s], in_=x_d[t]
                ).then_inc(s_load, 16)

        @block.tensor
        def _(tensor):
            tensor.wait_ge(s_w, 16 * G)
            for v in range(N_SUB):
                t, h = v // 2, v % 2
                g = t // N_LOC
                s = t % B
                if h == 0:
                    # RAW: load(t) landed.
                    tensor.wait_ge(s_load, 16 * (t + 1))
                if v >= 2:
                    # WAR: PSUM half h of sub v-2 fully cast out.
                    tensor.wait_ge(s_cpa, v - 1)
                    tensor.wait_ge(s_cpb, v - 1)
                for k in range(HALF // CHUNK):
                    mm = tensor.matmul(
                        ps[:, h, k * CHUNK:(k + 1) * CHUNK],
                        w_sb[:, g],
                        x_sb[:, s, h * HALF + k * CHUNK:
                                   h * HALF + (k + 1) * CHUNK],
                        start=True,
                        stop=True,
                    )
                    if k == HALF // CHUNK - 1:
                        mm.then_inc(s_mm, 1)

        @block.vector
        def _(vector):
            for v in range(N_SUB):
                t, h = v // 2, v % 2
                s = t % B
                # RAW: PE sub v done.
                vector.wait_ge(s_mm, v + 1)
                if h == 0 and t >= B:
                    # WAR: store(t-B) done reading this o slot.
                    vector.wait_ge(s_store, 16 * (t - B + 1))
                vector.tensor_copy(
                    out=o_sb[:, s, h * HALF:h * HALF + SPLIT],
                    in_=ps[:, h, 0:SPLIT],
                ).then_inc(s_cpa, 1)

        @block.scalar
        def _(scalar):
            for v in range(N_SUB):
                t, h = v // 2, v % 2
                s = t % B
                # RAW: PE sub v done.
                scalar.wait_ge(s_mm, v + 1)
                if h == 0 and t >= B:
                    # WAR: store(t-B) done reading this o slot.
                    scalar.wait_ge(s_store, 16 * (t - B + 1))
                scalar.copy(
                    o_sb[:, s, h * HALF + SPLIT:(h + 1) * HALF],
                    ps[:, h, SPLIT:HALF],
                ).then_inc(s_cpb, 1)
                if h == 1:
                    # RAW: all four casts of tile t done. Relaxed ordering:
                    # ACT's own casts above need the s_cpb wait too.
                    scalar.wait_ge(s_cpa, v + 1)
                    scalar.wait_ge(s_cpb, v + 1)
                    scalar.dma_start(
                        out=out_d[t], in_=o_sb[:, s]
                    ).then_inc(s_store, 16)

    _nc_cache["nc"] = nc
    return nc


def run_sharded(x_a, x_b, weights, **spmd_kwargs):
    """Shard, run on 8 cores, gather. Returns ((out_a, out_b), BassKernelResults)."""
    nc = _build()
    # Cast to fp16 FIRST (halves the bytes the interleave/transpose moves).
    # Tile t = g*N_LOC + n holds [(stream i, 64 channels), F] for channel
    # group g: partition p = i*64 + c_lo.
    xa = np.asarray(x_a).astype(FP16).reshape(N_CORES, N_LOC, G, 64, F)
    xb = np.asarray(x_b).astype(FP16).reshape(N_CORES, N_LOC, G, 64, F)
    x = np.stack([xa, xb], axis=3)              # [cores, n, g, i, 64, F]
    x = np.ascontiguousarray(
        x.transpose(0, 2, 1, 3, 4, 5)           # [cores, g, n, i, 64, F]
        .reshape(N_CORES, N_TILES, P, F)
    )
    # Block-diagonal mix matrices: W[g, i*64+c, o*64+c] = w[g*64+c, o, i].
    w = np.asarray(weights, dtype=np.float32).reshape(G, 64, 2, 2)
    wmat = np.zeros((G, P, P), dtype=np.float32)
    idx = np.arange(64)
    for i in range(2):
        for o in range(2):
            wmat[:, i * 64 + idx, o * 64 + idx] = w[:, idx, o, i]
    wmat = wmat.astype(FP16)
    in_maps = [{"x": x[i], "weights": wmat} for i in range(N_CORES)]
    res = run_bass_kernel_spmd(nc, in_maps, list(range(N_CORES)), **spmd_kwargs)
    out = np.stack([res.results[i]["out"] for i in range(N_CORES)])
    # out[core, t=(g,n), p=(o, c_lo), F] -> [2, N, C, H, W], upcast f32.
    out = out.reshape(N_CORES, G, N_LOC, 2, 64, F).astype(np.float32)
    out = out.transpose(3, 0, 2, 1, 4, 5).reshape(2, N, C, H, W)
    return (out[0], out[1]), res


def kernel(x_a, x_b, weights):
    (out_a, out_b), _ = run_sharded(x_a, x_b, weights)
    return out_a, out_b
